# revision 1
# baseline (speedup 1.0000x reference)
"""Causal self-attention (B=2, T=2048, D=1024, H=16, DH=64) on 8 trn2 cores.

Sharding: DP on batch (2) x TP on heads (4 heads/core). Each core computes
qkv for its heads from x[b]^T, RoPE, causal SDPA, and a partial row-parallel
output projection y^T [D, T]. Host sums TP partials, transposes, adds bias.

All matmuls run as float32r (fp32 rounded to 11 mantissa bits, full PE rate).
Everything on-device works in transposed orientation so no device transposes
are needed; the only cross-partition data movement is done on the PE (a
pair-swap permutation matmul for RoPE and a ones-outer-product broadcast for
the softmax normalizer).
"""
import sys

if "/opt/trn_rl_repo" not in sys.path:
    sys.path.insert(0, "/opt/trn_rl_repo")

import numpy as np

B, T, D = 2, 2048, 1024
H, DH = 16, 64
ROPE_BASE = 10000.0
NCORES = 8
TP = 4                # TP group size (cores per batch)
HL = H // TP          # heads per core = 4
CHUNK = 512           # t/q chunk
NCH = T // CHUNK      # 4
KT = 128              # k tile
NKT = T // KT         # 16
DIN = HL * DH         # 256 local head dims
NEG = -1.0e30
SCALE = 1.0 / float(np.sqrt(DH))

_compiled = None
_last_results = None


def _round_fp32r(x: np.ndarray) -> np.ndarray:
    u = np.ascontiguousarray(x, dtype=np.float32).view(np.uint32)
    u = (u + np.uint32(0x7FF) + ((u >> np.uint32(12)) & np.uint32(1))) & np.uint32(0xFFFFF000)
    return u.view(np.float32)


def _build(debug=False, stages=("qkv", "rope", "attn", "proj")):
    import concourse.bass as bass
    import concourse.mybir as mybir
    import concourse.tile as tile
    from concourse import bacc

    F32 = mybir.dt.float32
    F32R = mybir.dt.float32r
    ADD = mybir.AluOpType.add
    MULT = mybir.AluOpType.mult
    EXP = mybir.ActivationFunctionType.Exp

    nc = bacc.Bacc("TRN2", target_bir_lowering=False, num_devices=NCORES)

    xT = nc.dram_tensor("xT", [D, T], F32R, kind="ExternalInput")
    wqk = nc.dram_tensor("wqk", [D, 2 * DIN], F32R, kind="ExternalInput")
    wv = nc.dram_tensor("wv", [D, DIN], F32R, kind="ExternalInput")
    wproj = nc.dram_tensor("wproj", [DIN, D], F32R, kind="ExternalInput")
    bqk = nc.dram_tensor("bqk", [128, 4], F32, kind="ExternalInput")
    bv = nc.dram_tensor("bv", [128, DIN], F32, kind="ExternalInput")
    cos2 = nc.dram_tensor("cos2", [128, T], F32, kind="ExternalInput")
    sin2 = nc.dram_tensor("sin2", [128, T], F32, kind="ExternalInput")
    perm = nc.dram_tensor("perm", [128, 128], F32R, kind="ExternalInput")
    trimask = nc.dram_tensor("trimask", [128, 128], F32, kind="ExternalInput")
    vconst = nc.dram_tensor("vconst", [128, 64], F32R, kind="ExternalInput")
    yT = nc.dram_tensor("yT", [D, T], F32, kind="ExternalOutput")
    if debug:
        dbg_qk = nc.dram_tensor("dbg_qk", [128, 4, T], F32, kind="ExternalOutput")
        dbg_v = nc.dram_tensor("dbg_v", [128, NKT, 2, 192], F32, kind="ExternalOutput")
        if "attn" in stages:
            dbg_y = nc.dram_tensor("dbg_y", [128, 2, T], F32, kind="ExternalOutput")

    with tile.TileContext(nc) as tc:
        with tc.tile_pool(name="const", bufs=1) as constp, \
             tc.tile_pool(name="big", bufs=1) as bigp, \
             tc.tile_pool(name="xin", bufs=3) as xinp, \
             tc.tile_pool(name="ptile", bufs=4) as ptp, \
             tc.tile_pool(name="tmp", bufs=3) as tmpp, \
             tc.tile_pool(name="rsm", bufs=2) as rsmp, \
             tc.tile_pool(name="outs", bufs=3) as outsp, \
             tc.tile_pool(name="psmm", bufs=3, space="PSUM") as psmm, \
             tc.tile_pool(name="pss", bufs=3, space="PSUM") as pss, \
             tc.tile_pool(name="psav", bufs=2, space="PSUM") as psav:

            # ---- persistent SBUF tensors ----
            wqk_sb = constp.tile([128, 8, 2 * DIN], F32R)     # [p, din_o, f]
            wv_sb = constp.tile([128, 8, DIN], F32R)
            wproj_sb = constp.tile([128, 2, D], F32R)         # [p, din_tile, dout]
            bqk_sb = constp.tile([128, 4], F32)
            bv_sb = constp.tile([128, DIN], F32)
            cos_sb = constp.tile([128, T], F32)
            sin_sb = constp.tile([128, T], F32)
            perm_sb = constp.tile([128, 128], F32R)
            tri_sb = constp.tile([128, 128], F32)

            qk_sb = bigp.tile([128, 4, T], F32R)              # fb: q01,q23,k01,k23
            v_sb = bigp.tile([128, NKT, 2, 192], F32R)        # [t_p, kt, hp, cols]
            y_sb = bigp.tile([128, 2, T], F32R)               # y^T (din on partitions)

            # startup DMAs in consumption order: interleave wqk and x(chunk 0)
            # slices so the first qkv matmul starts after ~0.5MB, not ~9MB
            x_tiles = {}
            x_tiles[0] = xinp.tile([128, 8, CHUNK], F32R, tag="xchunk", name="x_c0")
            wqk3 = wqk[:].rearrange("(o p) f -> p o f", p=128)
            for o in range(8):
                nc.sync.dma_start(wqk_sb[:, o], wqk3[:, o])
                nc.sync.dma_start(x_tiles[0][:, o], xT[o * 128:(o + 1) * 128, 0:CHUNK])
            nc.sync.dma_start(bqk_sb[:], bqk[:])
            wv3 = wv[:].rearrange("(o p) f -> p o f", p=128)
            for o in range(8):
                nc.sync.dma_start(wv_sb[:, o], wv3[:, o])
            nc.sync.dma_start(bv_sb[:], bv[:])
            nc.sync.dma_start(perm_sb[:], perm[:])
            nc.sync.dma_start(cos_sb[:], cos2[:])
            nc.sync.dma_start(sin_sb[:], sin2[:])
            nc.sync.dma_start(tri_sb[:], trimask[:])
            nc.sync.dma_start(wproj_sb[:], wproj[:].rearrange("(o p) f -> p o f", p=128))

            # V layout per (kt, hp): [V_even(64) | ones(1) | zeros(63) | V_odd(64)]
            for kt in range(NKT):
                for hp in range(2):
                    nc.sync.dma_start(v_sb[:, kt, hp, 64:128], vconst[:])

            do_s = any(s in stages for s in ("attn", "attn_av", "attn_s"))
            do_av = any(s in stages for s in ("attn", "attn_av"))
            do_norm = "attn" in stages

            def emit_qkv(c):
                cc = bass.ds(c * CHUNK, CHUNK)
                x_sb = x_tiles.pop(c)
                # q^T,k^T for this chunk: [f, t]
                for fb in range(4):
                    pq = psmm.tile([128, CHUNK], F32, tag="mm", name="pq")
                    for o in range(8):
                        nc.tensor.matmul(
                            pq[:], wqk_sb[:, o, fb * 128:(fb + 1) * 128], x_sb[:, o],
                            start=(o == 0), stop=(o == 7))
                    # bias add (per-partition) + round to fp32r
                    nc.vector.tensor_scalar_add(qk_sb[:, fb, cc], pq[:], bqk_sb[:, fb:fb + 1])
                # V for this chunk (natural layout)
                for tb in range(4):
                    kt = c * 4 + tb
                    pvfull = psmm.tile([128, CHUNK], F32, tag="mm", name="pvfull")
                    pv = pvfull[:, :DIN]
                    for o in range(8):
                        nc.tensor.matmul(
                            pv[:], x_sb[:, o, tb * 128:(tb + 1) * 128], wv_sb[:, o],
                            start=(o == 0), stop=(o == 7))
                    for l in range(HL):
                        off = 0 if l % 2 == 0 else 128
                        nc.vector.tensor_tensor(
                            v_sb[:, kt, l // 2, off:off + 64],
                            pv[:, l * 64:(l + 1) * 64],
                            bv_sb[:, l * 64:(l + 1) * 64], ADD)

            def emit_xload(c):
                if c < NCH:
                    x_tiles[c] = xinp.tile([128, 8, CHUNK], F32R, tag="xchunk",
                                           name=f"x_c{c}")
                    for o in range(8):
                        nc.sync.dma_start(x_tiles[c][:, o],
                                          xT[o * 128:(o + 1) * 128,
                                             c * CHUNK:(c + 1) * CHUNK])

            def emit_rope(c):
                if "rope" not in stages:
                    return
                cc = bass.ds(c * CHUNK, CHUNK)
                for fb in range(4):
                    pp = psmm.tile([128, CHUNK], F32, tag="mm", name="pp")
                    nc.tensor.matmul(pp[:], perm_sb[:], qk_sb[:, fb, cc], start=True, stop=True)
                    swapped = tmpp.tile([128, CHUNK], F32, tag="rope")
                    nc.vector.tensor_tensor(swapped[:], pp[:], sin_sb[:, cc], MULT)
                    nc.vector.tensor_tensor(qk_sb[:, fb, cc], qk_sb[:, fb, cc], cos_sb[:, cc], MULT)
                    nc.vector.tensor_tensor(qk_sb[:, fb, cc], qk_sb[:, fb, cc], swapped[:], ADD)

            def emit_attn(c):
                # attention for q-chunk c, heads in even/odd pairs: even head
                # S-MMs use PE rows 0-63, odd rows 64-127 — adjacent MMs land
                # in different row-groups and run concurrently.
                cc = bass.ds(c * CHUNK, CHUNK)
                nkt_c = 4 * c + 4
                for hp in range(2 if do_s else 0):
                    pavs, p_tiles = [], []
                    if do_av:
                        for par in range(2):
                            pav = psav.tile([128, CHUNK], F32, tag="av", name=f"pav{par}")
                            pavs.append(pav)
                    for kt in range(nkt_c):
                        i = kt - 4 * c  # >=0 on diagonal tiles
                        col0 = 128 * i if i >= 0 else 0
                        pts = []
                        for par in range(2):  # par=0 even head, par=1 odd head
                            base = 64 * par
                            ps = pss.tile([128, CHUNK], F32, tag="s", name=f"ps{par}")
                            nc.tensor.matmul(
                                ps[:], qk_sb[base:base + 64, 2 + hp, kt * 128:(kt + 1) * 128],
                                qk_sb[base:base + 64, hp, cc], start=True, stop=True)
                            pts.append(ps)
                        for par in range(2):
                            pt = ptp.tile([128, CHUNK], F32R, tag="p", name=f"pt{par}")
                            nc.scalar.activation(pt[:, col0:], pts[par][:, col0:], EXP,
                                                 bias=0.0, scale=SCALE)
                            if i >= 0:
                                # zero k>q entries of the diagonal block (0/1 mask)
                                nc.vector.tensor_tensor(
                                    pt[:, col0:col0 + 128], pt[:, col0:col0 + 128],
                                    tri_sb[:], MULT)
                            p_tiles.append(pt)
                    for kt in range(nkt_c if do_av else 0):
                        i = kt - 4 * c
                        col0 = 128 * i if i >= 0 else 0
                        for par in range(2):
                            pt = p_tiles[2 * kt + par]
                            nout = 65 if par == 0 else 128
                            voff = 0 if par == 0 else 64
                            nc.tensor.matmul(
                                pavs[par][:nout, col0:], v_sb[:, kt, hp, voff:voff + nout],
                                pt[:, col0:], start=(kt == 0), stop=(kt == nkt_c - 1),
                                skip_group_check=True)
                    if not do_norm:
                        continue
                    for par in range(2):
                        # normalize: recip of sums row + partition-broadcast via
                        # DVE stream_shuffle, then scale y rows
                        pav, base = pavs[par], 64 * par
                        srow = 64 if par == 0 else 0
                        yrows = 0 if par == 0 else 64
                        rr_sb = rsmp.tile([128, CHUNK], F32, tag="r")
                        nc.vector.reciprocal(rr_sb[srow:srow + 1, :], pav[srow:srow + 1, :])
                        bc_sb = rsmp.tile([128, CHUNK], F32, tag="bcs")
                        src32 = rr_sb[srow:srow + 32, :]
                        nc.vector.stream_shuffle(bc_sb[base:base + 32, :], src32, [0] * 32)
                        nc.vector.stream_shuffle(bc_sb[base + 32:base + 64, :], src32, [0] * 32)
                        nc.vector.tensor_tensor(
                            y_sb[base:base + 64, hp, cc], pav[yrows:yrows + 64, :],
                            bc_sb[base:base + 64, :], MULT)

            def emit_proj(c):
                cc = bass.ds(c * CHUNK, CHUNK)
                for db in range(8 if "proj" in stages else 0):
                    pr = psmm.tile([128, CHUNK], F32, tag="mm", name="pr")
                    for pt2 in range(2):
                        nc.tensor.matmul(
                            pr[:], wproj_sb[:, pt2, db * 128:(db + 1) * 128],
                            y_sb[:, pt2, cc], start=(pt2 == 0), stop=(pt2 == 1))
                    o_sb = outsp.tile([128, CHUNK], F32, tag="o")
                    nc.scalar.copy(o_sb[:], pr[:])
                    nc.sync.dma_start(yT[db * 128:(db + 1) * 128, cc], o_sb[:])

            # software pipeline: run chunk c+1's qkv on PE while chunk c's
            # final normalize (DVE) completes, then proj(c)
            emit_qkv(0)
            emit_xload(1)
            emit_rope(0)
            for c in range(NCH):
                emit_attn(c)
                if c + 1 < NCH:
                    emit_xload(c + 2)
                    emit_qkv(c + 1)
                emit_proj(c)
                if c + 1 < NCH:
                    emit_rope(c + 1)

            if debug:
                nc.sync.dma_start(dbg_qk[:], qk_sb[:].bitcast(F32))
                nc.sync.dma_start(dbg_v[:], v_sb[:].bitcast(F32))
                if "attn" in stages:
                    nc.sync.dma_start(dbg_y[:], y_sb[:].bitcast(F32))

    nc.finalize()
    return nc


def _host_inputs(x, Wqkv, bqkv, Wproj):
    """Per-core input maps. Core c: batch c//TP, heads [4*(c%TP), 4*(c%TP)+4)."""
    # RoPE tables in ^T layout, rows = head-local dim d (pattern repeats each 64)
    d = np.arange(64)
    inv_freq = 1.0 / (ROPE_BASE ** (np.arange(0, DH, 2, dtype=np.float64) / DH))  # [32]
    ang = np.arange(T, dtype=np.float64)[None, :] * inv_freq[d // 2][:, None]     # [64, T]
    cos64 = np.cos(ang)
    sin64 = np.sin(ang) * np.where(d % 2 == 0, -1.0, 1.0)[:, None]
    cos2 = np.tile(cos64, (2, 1)).astype(np.float32)
    sin2 = np.tile(sin64, (2, 1)).astype(np.float32)

    perm = np.zeros((128, 128), np.float32)
    perm[np.arange(128) ^ 1, np.arange(128)] = 1.0

    ki, qi = np.meshgrid(np.arange(128), np.arange(128), indexing="ij")
    trimask = np.where(ki <= qi, 1.0, 0.0).astype(np.float32)

    vconst_np = np.zeros((128, 64), np.float32)
    vconst_np[:, 0] = 1.0

    Wq, Wk, Wv = Wqkv[:, :D], Wqkv[:, D:2 * D], Wqkv[:, 2 * D:]
    bq, bk, bvv = bqkv[:D], bqkv[D:2 * D], bqkv[2 * D:]

    maps = []
    for core in range(NCORES):
        b, r = core // TP, core % TP
        sl = slice(r * DIN, (r + 1) * DIN)
        wqk_c = np.concatenate([Wq[:, sl], Wk[:, sl]], axis=1)
        bqk_c = np.concatenate([bq[sl], bk[sl]]).astype(np.float32)
        maps.append({
            "xT": _round_fp32r(x[b].T),
            "wqk": _round_fp32r(wqk_c),
            "wv": _round_fp32r(Wv[:, sl]),
            "wproj": _round_fp32r(Wproj[sl, :]),
            "bqk": np.ascontiguousarray(bqk_c.reshape(4, 128).T),
            "bv": np.broadcast_to(bvv[sl].astype(np.float32), (128, DIN)).copy(),
            "cos2": cos2,
            "sin2": sin2,
            "perm": _round_fp32r(perm),
            "trimask": trimask,
            "vconst": vconst_np,
        })
    return maps


def kernel(x, Wqkv, bqkv, Wproj, bproj):
    global _compiled, _last_results
    from concourse.bass_utils import run_bass_kernel_spmd

    if _compiled is None:
        _compiled = _build()
    nc = _compiled

    maps = _host_inputs(
        np.asarray(x, np.float32), np.asarray(Wqkv, np.float32),
        np.asarray(bqkv, np.float32), np.asarray(Wproj, np.float32))
    res = run_bass_kernel_spmd(nc, maps, core_ids=list(range(NCORES)))
    _last_results = res
    out = np.empty((B, T, D), np.float32)
    for b in range(B):
        acc = np.zeros((D, T), np.float64)
        for r in range(TP):
            acc += res.results[b * TP + r]["yT"]
        out[b] = acc.T + np.asarray(bproj, np.float64)[None, :]
    return out



# revision 30
# speedup vs baseline: 1.5787x; 1.5787x over previous
"""Causal self-attention (B=2, T=2048, D=1024, H=16, DH=64) on 8 trn2 cores.

Sharding: DP on batch (2) x TP on heads (4 heads/core). Each core computes
qkv for its heads from x[b]^T, RoPE, causal SDPA, and a partial row-parallel
output projection y^T [D, T] (f16, x1024 scale). Host sums TP partials,
rescales, transposes, adds bias.

v4 precision/engine economics (TimelineSim cost model):
- matmul cost = moving rows only; fp8e4 DoubleRow (2 K-slices/pass) is the
  only fast mode, but a single fp8 quantization injects ~3% relative error
  into a dot product (it does NOT average out), which blows the 2e-2 budget.
  QKV therefore runs fp8 DR with host-side error-feedback residuals
  (x8@W8 + xr8@W8 + x8@Wr8 -> ~0.4% error at 3/4 the bf16 cost); S, AV and
  proj run bf16 (1.0 cyc/row, no free-size penalty).
- Weights scaled x32 on host so 0.02-scale values stay in e4m3 normal range;
  S carries x1024 folded into the exp scale; v carries x32 into y and proj
  another x32 -> one final /1024 on host.
- softmax exp: one ACT instruction per PAIR of k-tiles over a 2-bank PSUM
  span, bf16 out; causal mask = 0/1 bf16 multiply on GpSimd (GpSimd cannot
  touch PSUM; only SBUF-resident work goes there). Normalizer Z rides row
  64/0 of the AV psum via a ones-column in V.
- rope pair-swap via one 128-partition stream_shuffle (mask applies per
  32-quadrant); bf16 tensors get the DVE 2x mode.
- 1/Z broadcast via PE ones-outer-product; DMAs batched (x loads, one f16
  y^T store per db+chunk, V-const regions via memset).
"""
import sys

if "/opt/trn_rl_repo" not in sys.path:
    sys.path.insert(0, "/opt/trn_rl_repo")

import numpy as np
import ml_dtypes

F8NP = ml_dtypes.float8_e4m3
BF16NP = ml_dtypes.bfloat16

B, T, D = 2, 2048, 1024
H, DH = 16, 64
ROPE_BASE = 10000.0
NCORES = 8
TP = 4                # TP group size (cores per batch)
HL = H // TP          # heads per core = 4
CHUNK = 512           # t/q chunk
NCH = T // CHUNK      # 4
NKT = T // 128        # 16 k-tiles
NJ = NKT // 2         # k-tile pairs = 8
DIN = HL * DH         # 256 local head dims
WS = 32.0             # host weight scale (Wqkv, Wv, Wproj)
SCALE = 1.0 / float(np.sqrt(DH))
EXP_SCALE = SCALE / (WS * WS)         # S carries WS^2
OUT_DIV = float(WS * WS)              # host divides final output

_compiled = None
_last_results = None


def _round_fp32r(x: np.ndarray) -> np.ndarray:
    u = np.ascontiguousarray(x, dtype=np.float32).view(np.uint32)
    u = (u + np.uint32(0x7FF) + ((u >> np.uint32(12)) & np.uint32(1))) & np.uint32(0xFFFFF000)
    return u.view(np.float32)


def _build(debug=False):
    import concourse.bass as bass
    import concourse.mybir as mybir
    import concourse.tile as tile
    from concourse import bacc

    F32 = mybir.dt.float32
    F32R = mybir.dt.float32r
    F16 = mybir.dt.float16
    BF16 = mybir.dt.bfloat16
    F8 = mybir.dt.float8e4
    ADD = mybir.AluOpType.add
    MULT = mybir.AluOpType.mult
    EXP = mybir.ActivationFunctionType.Exp
    DR = mybir.MatmulPerfMode.DoubleRow
    SWAP_MASK = [i ^ 1 for i in range(32)]

    nc = bacc.Bacc("TRN2", target_bir_lowering=False, num_devices=NCORES)

    xT8 = nc.dram_tensor("xT8", [D, T], F8, kind="ExternalInput")
    xTr8 = nc.dram_tensor("xTr8", [D, T], F8, kind="ExternalInput")
    wqk8 = nc.dram_tensor("wqk8", [D, 2 * DIN], F8, kind="ExternalInput")
    wqkr8 = nc.dram_tensor("wqkr8", [D, 2 * DIN], F8, kind="ExternalInput")
    wv8 = nc.dram_tensor("wv8", [D, DIN], F8, kind="ExternalInput")
    wvr8 = nc.dram_tensor("wvr8", [D, DIN], F8, kind="ExternalInput")
    wproj16 = nc.dram_tensor("wproj16", [DIN, D], BF16, kind="ExternalInput")
    bqk = nc.dram_tensor("bqk", [128, 4], F32, kind="ExternalInput")
    bv = nc.dram_tensor("bv", [128, DIN], F32, kind="ExternalInput")
    cos2 = nc.dram_tensor("cos2", [128, T], BF16, kind="ExternalInput")
    sin2 = nc.dram_tensor("sin2", [128, T], BF16, kind="ExternalInput")
    tri16 = nc.dram_tensor("tri16", [128, 128], BF16, kind="ExternalInput")
    ones64 = nc.dram_tensor("ones64", [1, 128], F32R, kind="ExternalInput")
    yT = nc.dram_tensor("yT", [D, T], F16, kind="ExternalOutput")

    with tile.TileContext(nc) as tc:
        with tc.tile_pool(name="const", bufs=1) as constp, \
             tc.tile_pool(name="big", bufs=1) as bigp, \
             tc.tile_pool(name="p8p", bufs=4) as p8p, \
             tc.tile_pool(name="tmp", bufs=3) as tmpp, \
             tc.tile_pool(name="rsm", bufs=2) as rsmp, \
             tc.tile_pool(name="outs", bufs=2) as outsp, \
             tc.tile_pool(name="psmm", bufs=2, space="PSUM") as psmm, \
             tc.tile_pool(name="pss", bufs=2, space="PSUM") as pss, \
             tc.tile_pool(name="psav", bufs=2, space="PSUM") as psav:

            # ---- persistent SBUF tensors ----
            wqk_sb = constp.tile([128, 8, 2 * DIN], F8)       # [p, o, f]
            wqkr_sb = constp.tile([128, 8, 2 * DIN], F8)
            wv_sb = constp.tile([128, 8, DIN], F8)
            wvr_sb = constp.tile([128, 8, DIN], F8)
            wproj_sb = constp.tile([128, 2, D], BF16)         # [p, pt2, dout]
            bqk_sb = constp.tile([128, 4], F32)
            bv_sb = constp.tile([128, DIN], F32)
            cos_sb = constp.tile([128, T], BF16)
            sin_sb = constp.tile([128, T], BF16)
            tri_sb = constp.tile([128, 128], BF16)
            ones_sb = constp.tile([1, 128], F32R)

            x_sb = bigp.tile([128, 8, T], F8)                 # all of x^T (fp8)
            xr_sb = bigp.tile([128, 8, T], F8)                # fp8 residual of x^T
            qk_sb = bigp.tile([128, 4, T], BF16)              # fb: q01,q23,k01,k23
            v16_sb = bigp.tile([128, NKT, 2, 192], BF16)      # [kpos, kt, hp, cols]
            y16_sb = bigp.tile([128, 2, T], BF16)             # y^T bf16 (x32 scale)

            # startup DMAs, batched; chunk-0 deps first so qkv can start early
            def xload(c):
                for t_sb, t_dr in ((x_sb, xT8), (xr_sb, xTr8)):
                    nc.sync.dma_start(
                        t_sb[:, :, c * CHUNK:(c + 1) * CHUNK],
                        t_dr[:].rearrange("(o p) t -> p o t", p=128)[
                            :, :, c * CHUNK:(c + 1) * CHUNK])

            xload(0)
            wqk4 = wqk8[:].rearrange("(o p) (fb f) -> p o fb f", p=128, f=128)
            wqkr4 = wqkr8[:].rearrange("(o p) (fb f) -> p o fb f", p=128, f=128)
            wqk_sb4 = wqk_sb[:].rearrange("p o (fb f) -> p o fb f", f=128)
            wqkr_sb4 = wqkr_sb[:].rearrange("p o (fb f) -> p o fb f", f=128)
            for fb in (0, 2):
                nc.sync.dma_start(wqk_sb4[:, :, fb], wqk4[:, :, fb])
                nc.sync.dma_start(wqkr_sb4[:, :, fb], wqkr4[:, :, fb])
            nc.sync.dma_start(bqk_sb[:], bqk[:])
            nc.sync.dma_start(cos_sb[:], cos2[:])
            nc.sync.dma_start(sin_sb[:], sin2[:])
            nc.sync.dma_start(tri_sb[:], tri16[:])
            for fb in (1, 3):
                nc.sync.dma_start(wqk_sb4[:, :, fb], wqk4[:, :, fb])
                nc.sync.dma_start(wqkr_sb4[:, :, fb], wqkr4[:, :, fb])
            nc.sync.dma_start(wv_sb[:], wv8[:].rearrange("(o p) f -> p o f", p=128))
            nc.sync.dma_start(wvr_sb[:], wvr8[:].rearrange("(o p) f -> p o f", p=128))
            nc.sync.dma_start(bv_sb[:], bv[:])
            xload(1)
            nc.sync.dma_start(ones_sb[:], ones64[:])
            nc.sync.dma_start(wproj_sb[:], wproj16[:].rearrange("(o p) f -> p o f", p=128))
            xload(2)
            xload(3)

            # V const regions: col 64 = 1.0, cols 65:128 = 0.0 (per kt, hp)
            nc.gpsimd.memset(v16_sb[:, :, :, 64:65], 1.0)
            nc.gpsimd.memset(v16_sb[:, :, :, 65:128], 0.0)

            # dummy EXP so the ACT table loads during startup DMAs
            scratch = tmpp.tile([128, 1], F32, tag="warm")
            nc.scalar.activation(scratch[:], bqk_sb[:, 0:1], EXP, bias=0.0, scale=0.0)

            def emit_qkv_qk(c, fbs=(0, 1, 2, 3)):
                cc = bass.ds(c * CHUNK, CHUNK)
                xc, xrc = x_sb[:, :, cc], xr_sb[:, :, cc]
                # q^T,k^T: fp8 DR with residual terms, accumulated in PSUM
                for fb in fbs:
                    fsl = bass.ds(fb * 128, 128)
                    pq = psmm.tile([128, CHUNK], F32, tag="mm", name="pq")
                    for j in range(4):
                        nc.tensor.matmul(pq[:], wqk_sb[:, 2 * j:2 * j + 2, fsl],
                                         xc[:, 2 * j:2 * j + 2],
                                         start=(j == 0), stop=False, perf_mode=DR)
                    for j in range(4):
                        nc.tensor.matmul(pq[:], wqk_sb[:, 2 * j:2 * j + 2, fsl],
                                         xrc[:, 2 * j:2 * j + 2],
                                         start=False, stop=False, perf_mode=DR)
                    for j in range(4):
                        nc.tensor.matmul(pq[:], wqkr_sb[:, 2 * j:2 * j + 2, fsl],
                                         xc[:, 2 * j:2 * j + 2],
                                         start=False, stop=(j == 3), perf_mode=DR)
                    nc.vector.tensor_scalar_add(qk_sb[:, fb, cc], pq[:], bqk_sb[:, fb:fb + 1])

            def emit_qkv_v(c):
                cc = bass.ds(c * CHUNK, CHUNK)
                # V for this chunk (natural layout [t, f])
                for tb in range(4):
                    kt = c * 4 + tb
                    tsl = bass.ds(tb * 128, 128)
                    pvfull = psmm.tile([128, CHUNK], F32, tag="mm", name="pvfull")
                    pv = pvfull[:, :DIN]
                    for j in range(4):
                        nc.tensor.matmul(pv[:], x_sb[:, 2 * j:2 * j + 2, cc][:, :, tsl],
                                         wv_sb[:, 2 * j:2 * j + 2],
                                         start=(j == 0), stop=False, perf_mode=DR)
                    for j in range(4):
                        nc.tensor.matmul(pv[:], xr_sb[:, 2 * j:2 * j + 2, cc][:, :, tsl],
                                         wv_sb[:, 2 * j:2 * j + 2],
                                         start=False, stop=False, perf_mode=DR)
                    for j in range(4):
                        nc.tensor.matmul(pv[:], x_sb[:, 2 * j:2 * j + 2, cc][:, :, tsl],
                                         wvr_sb[:, 2 * j:2 * j + 2],
                                         start=False, stop=(j == 3), perf_mode=DR)
                    for par in range(2):
                        # even heads l=0,2 -> hp 0,1 cols 0:64 ; odd l=1,3 -> 128:192
                        nc.vector.tensor_tensor(
                            v16_sb[:, kt, :, 128 * par:128 * par + 64],
                            pv[:].rearrange("p (l d) -> p l d", d=64)[:, par::2],
                            bv_sb[:].rearrange("p (l d) -> p l d", d=64)[:, par::2],
                            ADD)

            def emit_rope(c, fbs=(0, 1, 2, 3), fast=False):
                # fast=True keeps the whole chain on DVE (bf16 2x) to minimize
                # latency when other engines are idle (startup)
                eng2 = nc.vector if fast else nc.gpsimd
                cc = bass.ds(c * CHUNK, CHUNK)
                for fb in fbs:
                    swp = tmpp.tile([128, CHUNK], BF16, tag="rope")
                    nc.vector.stream_shuffle(swp[:], qk_sb[:, fb, cc], SWAP_MASK)
                    nc.vector.tensor_tensor(swp[:], swp[:], sin_sb[:, cc], MULT)
                    eng2.tensor_tensor(qk_sb[:, fb, cc], qk_sb[:, fb, cc],
                                       cos_sb[:, cc], MULT)
                    eng2.tensor_tensor(qk_sb[:, fb, cc], qk_sb[:, fb, cc],
                                       swp[:], ADD)

            def emit_attn_A(c, hp, par):
                """S pairs + EXP + mask -> p16 tile; returns the tile."""
                base = 64 * par
                nj_c = 2 * c + 2
                p16 = p8p.tile([128, NJ, 2, CHUNK], BF16, tag="p8", name=f"p16_{hp}{par}")
                for j in range(nj_c):
                    diag = (j >= 2 * c)
                    msp = 256 if (diag and j == 2 * c + 1) else 0
                    p2 = pss.tile([128, 2, CHUNK], F32, tag="s", name=f"s{j}")
                    for sl in range(2):
                        kt = 2 * j + sl
                        ms = 128 * (kt - 4 * c) if diag else 0
                        nc.tensor.matmul(
                            p2[:, sl, ms:],
                            qk_sb[base:base + 64, 2 + hp, kt * 128:(kt + 1) * 128],
                            qk_sb[base:base + 64, hp, bass.ds(c * CHUNK + ms, CHUNK - ms)],
                            start=True, stop=True)
                    nc.scalar.activation(p16[:, j, :, msp:], p2[:, :, msp:],
                                         EXP, bias=0.0, scale=EXP_SCALE)
                    if diag:
                        i0 = 2 * j - 4 * c  # 0 or 2
                        for sl in range(2):
                            col0 = 128 * (i0 + sl)
                            nc.gpsimd.tensor_tensor(
                                p16[:, j, sl, col0:col0 + 128],
                                p16[:, j, sl, col0:col0 + 128], tri_sb[:], MULT)
                            if sl == 1:
                                nc.gpsimd.memset(p16[:, j, 1, col0 - 128:col0], 0.0)
                return p16

            def emit_attn_B(c, hp, par, p16):
                """AV (bf16, per k-tile) + normalize -> y16."""
                cc = bass.ds(c * CHUNK, CHUNK)
                base = 64 * par
                nkt_c = 4 * c + 4
                nout = 65 if par == 0 else 128
                voff = 0 if par == 0 else 64
                pav = psav.tile([128, CHUNK], F32, tag="av", name=f"pav{par}")
                for kt in range(nkt_c):
                    i = kt - 4 * c
                    ms = 128 * i if i >= 0 else 0
                    nc.tensor.matmul(
                        pav[:nout, ms:], v16_sb[:, kt, hp, voff:voff + nout],
                        p16[:, kt // 2, kt % 2, ms:], start=(kt == 0),
                        stop=(kt == nkt_c - 1), skip_group_check=True)
                srow = 64 if par == 0 else 0
                yrows = 0 if par == 0 else 64
                rr_sb = rsmp.tile([1, CHUNK], F32R, tag="r")
                with nc.allow_low_precision(reason="1/Z for softmax, f32r is plenty"):
                    nc.vector.reciprocal(rr_sb[:], pav[srow:srow + 1, :])
                pbc = psmm.tile([128, CHUNK], F32, tag="mm", name="pbc")
                nc.tensor.matmul(pbc[:], ones_sb[:], rr_sb[:], start=True, stop=True)
                bc_sb = rsmp.tile([128, CHUNK], F32, tag="bc")
                nc.vector.tensor_copy(bc_sb[base:base + 64, :], pbc[base:base + 64, :])
                nc.vector.tensor_tensor(
                    y16_sb[base:base + 64, hp, cc], pav[yrows:yrows + 64, :],
                    bc_sb[base:base + 64, :], MULT)

            def emit_proj(c, tail=False):
                cc = bass.ds(c * CHUNK, CHUNK)
                o_sb = outsp.tile([128, 8, CHUNK], F16, tag="o")
                for db in range(8):
                    if tail and db % 2 == 1:
                        # EXP stream is done; reuse idle pss banks to widen the ring
                        pr = pss.tile([128, 2, CHUNK], F32, tag="s", name="prs")[:, 0]
                    else:
                        pr = psmm.tile([128, CHUNK], F32, tag="mm", name="pr")
                    for pt2 in range(2):
                        nc.tensor.matmul(
                            pr[:], wproj_sb[:, pt2, db * 128:(db + 1) * 128],
                            y16_sb[:, pt2, cc], start=(pt2 == 0), stop=(pt2 == 1))
                    if tail:
                        # latency-optimized: halves on both engines
                        nc.vector.tensor_copy(o_sb[:, db, 0:256], pr[:, 0:256])
                        nc.scalar.copy(o_sb[:, db, 256:], pr[:, 256:])
                    elif db % 2 == 0:
                        nc.vector.tensor_copy(o_sb[:, db], pr[:])
                    else:
                        nc.scalar.copy(o_sb[:, db], pr[:])
                    nc.sync.dma_start(
                        yT[db * 128:(db + 1) * 128, cc], o_sb[:, db])

            # software pipeline: attn A/B interleaved; qkv(c+1) and rope(c+1)
            # emitted early inside chunk c so chunk c+1's S can start the
            # moment chunk c's EXP stream drains; proj delayed by half a chunk.
            BLKS = [(0, 0), (0, 1), (1, 0), (1, 1)]
            emit_qkv_qk(0, (0, 2))
            emit_rope(0, (0, 2), fast=True)
            emit_qkv_qk(0, (1, 3))
            emit_rope(0, (1, 3), fast=True)
            emit_qkv_v(0)
            pend_proj = None
            for c in range(NCH):
                p8s = {}
                p8s[0] = emit_attn_A(c, *BLKS[0])
                if c + 1 < NCH:
                    emit_qkv_qk(c + 1, (0, 2))
                p8s[1] = emit_attn_A(c, *BLKS[1])
                if pend_proj is not None:
                    emit_proj(pend_proj)
                    pend_proj = None
                if c + 1 < NCH:
                    emit_rope(c + 1, (0, 2))
                p8s[2] = emit_attn_A(c, *BLKS[2])
                emit_attn_B(c, *BLKS[0], p8s.pop(0))
                if c + 1 < NCH:
                    emit_qkv_qk(c + 1, (1, 3))
                p8s[3] = emit_attn_A(c, *BLKS[3])
                emit_attn_B(c, *BLKS[1], p8s.pop(1))
                if c + 1 < NCH:
                    emit_rope(c + 1, (1, 3))
                emit_attn_B(c, *BLKS[2], p8s.pop(2))
                if c + 1 < NCH:
                    emit_qkv_v(c + 1)
                emit_attn_B(c, *BLKS[3], p8s.pop(3))
                pend_proj = c
            emit_proj(NCH - 1, tail=True)

    nc.finalize()
    return nc


def _host_inputs(x, Wqkv, bqkv, Wproj):
    """Per-core input maps. Core c: batch c//TP, heads [4*(c%TP), 4*(c%TP)+4)."""
    d = np.arange(64)
    inv_freq = 1.0 / (ROPE_BASE ** (np.arange(0, DH, 2, dtype=np.float64) / DH))  # [32]
    ang = np.arange(T, dtype=np.float64)[None, :] * inv_freq[d // 2][:, None]     # [64, T]
    cos64 = np.cos(ang)
    sin64 = np.sin(ang) * np.where(d % 2 == 0, -1.0, 1.0)[:, None]
    cos2 = np.tile(cos64, (2, 1)).astype(BF16NP)
    sin2 = np.tile(sin64, (2, 1)).astype(BF16NP)

    ki, qi = np.meshgrid(np.arange(128), np.arange(128), indexing="ij")
    tri16 = np.where(ki <= qi, 1.0, 0.0).astype(BF16NP)

    ones64 = _round_fp32r(np.ones((1, 128), np.float32))

    def f8_pair(a):
        a8 = a.astype(F8NP)
        ar8 = (a - a8.astype(np.float64)).astype(F8NP)
        return a8, ar8

    Wq, Wk, Wv = Wqkv[:, :D], Wqkv[:, D:2 * D], Wqkv[:, 2 * D:]
    bq, bk, bvv = bqkv[:D], bqkv[D:2 * D], bqkv[2 * D:]

    maps = []
    for core in range(NCORES):
        b, r = core // TP, core % TP
        sl = slice(r * DIN, (r + 1) * DIN)
        wqk_c = np.concatenate([Wq[:, sl], Wk[:, sl]], axis=1).astype(np.float64) * WS
        bqk_c = (np.concatenate([bq[sl], bk[sl]]) * WS).astype(np.float32)
        x8, xr8 = f8_pair(x[b].T.astype(np.float64))
        wqk8, wqkr8 = f8_pair(wqk_c)
        wv8, wvr8 = f8_pair(Wv[:, sl].astype(np.float64) * WS)
        maps.append({
            "xT8": x8, "xTr8": xr8,
            "wqk8": wqk8, "wqkr8": wqkr8,
            "wv8": wv8, "wvr8": wvr8,
            "wproj16": (Wproj[sl, :] * WS).astype(BF16NP),
            "bqk": np.ascontiguousarray(bqk_c.reshape(4, 128).T),
            "bv": np.broadcast_to((bvv[sl] * WS).astype(np.float32), (128, DIN)).copy(),
            "cos2": cos2,
            "sin2": sin2,
            "tri16": tri16,
            "ones64": ones64,
        })
    return maps


def kernel(x, Wqkv, bqkv, Wproj, bproj):
    global _compiled, _last_results
    from concourse.bass_utils import run_bass_kernel_spmd

    if _compiled is None:
        _compiled = _build()
    nc = _compiled

    maps = _host_inputs(
        np.asarray(x, np.float32), np.asarray(Wqkv, np.float32),
        np.asarray(bqkv, np.float32), np.asarray(Wproj, np.float32))
    res = run_bass_kernel_spmd(nc, maps, core_ids=list(range(NCORES)))
    _last_results = res
    out = np.empty((B, T, D), np.float32)
    for b in range(B):
        acc = np.zeros((D, T), np.float64)
        for r in range(TP):
            acc += res.results[b * TP + r]["yT"].astype(np.float64)
        out[b] = (acc / OUT_DIV).T + np.asarray(bproj, np.float64)[None, :]
    return out


# revision 41
# speedup vs baseline: 1.6272x; 1.0307x over previous
"""Causal self-attention (B=2, T=2048, D=1024, H=16, DH=64) on 8 trn2 cores.

Sharding: DP on batch (2) x TP on heads (4 heads/core). Each core computes
qkv for its heads from x[b]^T, RoPE, causal SDPA, and a partial row-parallel
output projection y^T [D, T] (f16, x1024 scale). Host sums TP partials,
rescales, transposes, adds bias.

v4 precision/engine economics (TimelineSim cost model):
- matmul cost = moving rows only; fp8e4 DoubleRow (2 K-slices/pass) is the
  only fast mode, but a single fp8 quantization injects ~3% relative error
  into a dot product (it does NOT average out), which blows the 2e-2 budget.
  QKV therefore runs fp8 DR with host-side error-feedback residuals
  (x8@W8 + xr8@W8 + x8@Wr8 -> ~0.4% error at 3/4 the bf16 cost); S, AV and
  proj run bf16 (1.0 cyc/row, no free-size penalty).
- Weights scaled x32 on host so 0.02-scale values stay in e4m3 normal range;
  S carries x1024 folded into the exp scale; v carries x32 into y and proj
  another x32 -> one final /1024 on host.
- softmax exp: one ACT instruction per PAIR of k-tiles over a 2-bank PSUM
  span, bf16 out; causal mask = 0/1 bf16 multiply on GpSimd (GpSimd cannot
  touch PSUM; only SBUF-resident work goes there). Normalizer Z rides row
  64/0 of the AV psum via a ones-column in V.
- rope pair-swap via one 128-partition stream_shuffle (mask applies per
  32-quadrant); bf16 tensors get the DVE 2x mode.
- 1/Z broadcast via PE ones-outer-product; DMAs batched (x loads, one f16
  y^T store per db+chunk, V-const regions via memset).
"""
import sys

if "/opt/trn_rl_repo" not in sys.path:
    sys.path.insert(0, "/opt/trn_rl_repo")

import numpy as np
import ml_dtypes

F8NP = ml_dtypes.float8_e4m3
BF16NP = ml_dtypes.bfloat16

B, T, D = 2, 2048, 1024
H, DH = 16, 64
ROPE_BASE = 10000.0
NCORES = 8
TP = 4                # TP group size (cores per batch)
HL = H // TP          # heads per core = 4
CHUNK = 512           # t/q chunk
NCH = T // CHUNK      # 4
NKT = T // 128        # 16 k-tiles
NJ = NKT // 2         # k-tile pairs = 8
DIN = HL * DH         # 256 local head dims
WS = 32.0             # host weight scale (Wqkv, Wv, Wproj)
SCALE = 1.0 / float(np.sqrt(DH))
EXP_SCALE = SCALE / (WS * WS)         # S carries WS^2
OUT_DIV = float(WS * WS)              # host divides final output

_compiled = None
_last_results = None


def _round_fp32r(x: np.ndarray) -> np.ndarray:
    u = np.ascontiguousarray(x, dtype=np.float32).view(np.uint32)
    u = (u + np.uint32(0x7FF) + ((u >> np.uint32(12)) & np.uint32(1))) & np.uint32(0xFFFFF000)
    return u.view(np.float32)


def _build(debug=False):
    import concourse.bass as bass
    import concourse.mybir as mybir
    import concourse.tile as tile
    from concourse import bacc

    F32 = mybir.dt.float32
    F32R = mybir.dt.float32r
    F16 = mybir.dt.float16
    BF16 = mybir.dt.bfloat16
    F8 = mybir.dt.float8e4
    ADD = mybir.AluOpType.add
    MULT = mybir.AluOpType.mult
    EXP = mybir.ActivationFunctionType.Exp
    DR = mybir.MatmulPerfMode.DoubleRow
    SWAP_MASK = [i ^ 1 for i in range(32)]

    nc = bacc.Bacc("TRN2", target_bir_lowering=False, num_devices=NCORES)

    xT8 = nc.dram_tensor("xT8", [D, T], F8, kind="ExternalInput")
    xTr8 = nc.dram_tensor("xTr8", [D, T], F8, kind="ExternalInput")
    wqk8 = nc.dram_tensor("wqk8", [D, 2 * DIN], F8, kind="ExternalInput")
    wqkr8 = nc.dram_tensor("wqkr8", [D, 2 * DIN], F8, kind="ExternalInput")
    wv8 = nc.dram_tensor("wv8", [D, DIN], F8, kind="ExternalInput")
    wvr8 = nc.dram_tensor("wvr8", [D, DIN], F8, kind="ExternalInput")
    wproj16 = nc.dram_tensor("wproj16", [DIN, D], BF16, kind="ExternalInput")
    bqk = nc.dram_tensor("bqk", [128, 4], F32, kind="ExternalInput")
    bv = nc.dram_tensor("bv", [128, DIN], F32, kind="ExternalInput")
    cos2 = nc.dram_tensor("cos2", [128, T], BF16, kind="ExternalInput")
    sin2 = nc.dram_tensor("sin2", [128, T], BF16, kind="ExternalInput")
    tri16 = nc.dram_tensor("tri16", [128, 128], BF16, kind="ExternalInput")
    ones64 = nc.dram_tensor("ones64", [1, 128], F32R, kind="ExternalInput")
    yT = nc.dram_tensor("yT", [D, T], F16, kind="ExternalOutput")

    with tile.TileContext(nc) as tc:
        with tc.tile_pool(name="const", bufs=1) as constp, \
             tc.tile_pool(name="big", bufs=1) as bigp, \
             tc.tile_pool(name="p8p", bufs=4) as p8p, \
             tc.tile_pool(name="tmp", bufs=3) as tmpp, \
             tc.tile_pool(name="rsm", bufs=2) as rsmp, \
             tc.tile_pool(name="outs", bufs=2) as outsp, \
             tc.tile_pool(name="psmm", bufs=2, space="PSUM") as psmm, \
             tc.tile_pool(name="pss", bufs=2, space="PSUM") as pss, \
             tc.tile_pool(name="psav", bufs=2, space="PSUM") as psav:

            # ---- persistent SBUF tensors ----
            wqk_sb = constp.tile([128, 8, 2 * DIN], F8)       # [p, o, f]
            wqkr_sb = constp.tile([128, 8, 2 * DIN], F8)
            wv_sb = constp.tile([128, 8, DIN], F8)
            wvr_sb = constp.tile([128, 8, DIN], F8)
            wproj_sb = constp.tile([128, 2, D], BF16)         # [p, pt2, dout]
            bqk_sb = constp.tile([128, 4], F32)
            bv_sb = constp.tile([128, DIN], F32)
            cos_sb = constp.tile([128, T], BF16)
            sin_sb = constp.tile([128, T], BF16)
            tri_sb = constp.tile([128, 128], BF16)
            ones_sb = constp.tile([1, 128], F32R)

            x_sb = bigp.tile([128, 8, T], F8)                 # all of x^T (fp8)
            xr_sb = bigp.tile([128, 8, T], F8)                # fp8 residual of x^T
            qk_sb = bigp.tile([128, 4, T], BF16)              # fb: q01,q23,k01,k23
            v16_sb = bigp.tile([128, NKT, 2, 192], BF16)      # [kpos, kt, hp, cols]
            y16_sb = bigp.tile([128, 2, T], BF16)             # y^T bf16 (x32 scale)

            # startup DMAs, batched; chunk-0 deps first so qkv can start early
            def xload(c):
                for t_sb, t_dr in ((x_sb, xT8), (xr_sb, xTr8)):
                    nc.sync.dma_start(
                        t_sb[:, :, c * CHUNK:(c + 1) * CHUNK],
                        t_dr[:].rearrange("(o p) t -> p o t", p=128)[
                            :, :, c * CHUNK:(c + 1) * CHUNK])

            wqk4 = wqk8[:].rearrange("(o p) (fb f) -> p o fb f", p=128, f=128)
            wqkr4 = wqkr8[:].rearrange("(o p) (fb f) -> p o fb f", p=128, f=128)
            wqk_sb4 = wqk_sb[:].rearrange("p o (fb f) -> p o fb f", f=128)
            wqkr_sb4 = wqkr_sb[:].rearrange("p o (fb f) -> p o fb f", f=128)
            # DMAs ordered by first-use time; first-matmul deps split small
            xT3 = xT8[:].rearrange("(o p) t -> p o t", p=128)
            xTr3 = xTr8[:].rearrange("(o p) t -> p o t", p=128)
            nc.sync.dma_start(x_sb[:, 0:2, 0:CHUNK], xT3[:, 0:2, 0:CHUNK])
            nc.sync.dma_start(wqk_sb4[:, :, 0], wqk4[:, :, 0])
            nc.sync.dma_start(x_sb[:, 2:8, 0:CHUNK], xT3[:, 2:8, 0:CHUNK])
            nc.sync.dma_start(wqk_sb4[:, :, 2], wqk4[:, :, 2])
            nc.sync.dma_start(xr_sb[:, :, 0:CHUNK], xTr3[:, :, 0:CHUNK])
            for fb in (0, 2):
                nc.sync.dma_start(wqkr_sb4[:, :, fb], wqkr4[:, :, fb])
            nc.sync.dma_start(bqk_sb[:], bqk[:])
            nc.sync.dma_start(cos_sb[:], cos2[:])
            nc.sync.dma_start(sin_sb[:], sin2[:])
            for fb in (1, 3):
                nc.sync.dma_start(wqk_sb4[:, :, fb], wqk4[:, :, fb])
                nc.sync.dma_start(wqkr_sb4[:, :, fb], wqkr4[:, :, fb])
            xload(1)
            nc.sync.dma_start(wv_sb[:], wv8[:].rearrange("(o p) f -> p o f", p=128))
            nc.sync.dma_start(wvr_sb[:], wvr8[:].rearrange("(o p) f -> p o f", p=128))
            nc.sync.dma_start(bv_sb[:], bv[:])
            nc.sync.dma_start(tri_sb[:], tri16[:])
            nc.sync.dma_start(ones_sb[:], ones64[:])
            nc.sync.dma_start(wproj_sb[:], wproj16[:].rearrange("(o p) f -> p o f", p=128))
            xload(2)
            xload(3)

            # V const regions: col 64 = 1.0, cols 65:128 = 0.0 (per kt, hp)
            nc.gpsimd.memset(v16_sb[:, :, :, 64:65], 1.0)
            nc.gpsimd.memset(v16_sb[:, :, :, 65:128], 0.0)

            # dummy EXP so the ACT table loads during startup DMAs
            scratch = tmpp.tile([128, 1], F32, tag="warm")
            nc.scalar.activation(scratch[:], bqk_sb[:, 0:1], EXP, bias=0.0, scale=0.0)

            def emit_qkv_qk(c, fbs=(0, 1, 2, 3)):
                cc = bass.ds(c * CHUNK, CHUNK)
                xc, xrc = x_sb[:, :, cc], xr_sb[:, :, cc]
                # q^T,k^T: fp8 DR with residual terms, accumulated in PSUM
                for fb in fbs:
                    fsl = bass.ds(fb * 128, 128)
                    pq = psmm.tile([128, CHUNK], F32, tag="mm", name="pq")
                    for j in range(4):
                        nc.tensor.matmul(pq[:], wqk_sb[:, 2 * j:2 * j + 2, fsl],
                                         xc[:, 2 * j:2 * j + 2],
                                         start=(j == 0), stop=False, perf_mode=DR)
                    for j in range(4):
                        nc.tensor.matmul(pq[:], wqk_sb[:, 2 * j:2 * j + 2, fsl],
                                         xrc[:, 2 * j:2 * j + 2],
                                         start=False, stop=False, perf_mode=DR)
                    for j in range(4):
                        nc.tensor.matmul(pq[:], wqkr_sb[:, 2 * j:2 * j + 2, fsl],
                                         xc[:, 2 * j:2 * j + 2],
                                         start=False, stop=(j == 3), perf_mode=DR)
                    nc.vector.tensor_scalar_add(qk_sb[:, fb, cc], pq[:], bqk_sb[:, fb:fb + 1])

            def emit_qkv_v(c):
                cc = bass.ds(c * CHUNK, CHUNK)
                # V for this chunk (natural layout [t, f])
                for tb in range(4):
                    kt = c * 4 + tb
                    tsl = bass.ds(tb * 128, 128)
                    pvfull = psmm.tile([128, CHUNK], F32, tag="mm", name="pvfull")
                    pv = pvfull[:, :DIN]
                    for j in range(4):
                        nc.tensor.matmul(pv[:], x_sb[:, 2 * j:2 * j + 2, cc][:, :, tsl],
                                         wv_sb[:, 2 * j:2 * j + 2],
                                         start=(j == 0), stop=False, perf_mode=DR)
                    for j in range(4):
                        nc.tensor.matmul(pv[:], xr_sb[:, 2 * j:2 * j + 2, cc][:, :, tsl],
                                         wv_sb[:, 2 * j:2 * j + 2],
                                         start=False, stop=False, perf_mode=DR)
                    for j in range(4):
                        nc.tensor.matmul(pv[:], x_sb[:, 2 * j:2 * j + 2, cc][:, :, tsl],
                                         wvr_sb[:, 2 * j:2 * j + 2],
                                         start=False, stop=(j == 3), perf_mode=DR)
                    for par in range(2):
                        # even heads l=0,2 -> hp 0,1 cols 0:64 ; odd l=1,3 -> 128:192
                        nc.vector.tensor_tensor(
                            v16_sb[:, kt, :, 128 * par:128 * par + 64],
                            pv[:].rearrange("p (l d) -> p l d", d=64)[:, par::2],
                            bv_sb[:].rearrange("p (l d) -> p l d", d=64)[:, par::2],
                            ADD)

            def emit_rope(c, fbs=(0, 1, 2, 3), fast=False):
                # fast=True keeps the whole chain on DVE (bf16 2x) to minimize
                # latency when other engines are idle (startup)
                eng2 = nc.vector if fast else nc.gpsimd
                cc = bass.ds(c * CHUNK, CHUNK)
                for fb in fbs:
                    swp = tmpp.tile([128, CHUNK], BF16, tag="rope")
                    nc.vector.stream_shuffle(swp[:], qk_sb[:, fb, cc], SWAP_MASK)
                    nc.vector.tensor_tensor(swp[:], swp[:], sin_sb[:, cc], MULT)
                    eng2.tensor_tensor(qk_sb[:, fb, cc], qk_sb[:, fb, cc],
                                       cos_sb[:, cc], MULT)
                    eng2.tensor_tensor(qk_sb[:, fb, cc], qk_sb[:, fb, cc],
                                       swp[:], ADD)

            def emit_attn_A(c, hp, par):
                """S pairs + EXP + mask -> p16 tile; returns the tile."""
                base = 64 * par
                nj_c = 2 * c + 2
                p16 = p8p.tile([128, NJ, 2, CHUNK], BF16, tag="p8", name=f"p16_{hp}{par}")
                for j in range(nj_c):
                    diag = (j >= 2 * c)
                    msp = 256 if (diag and j == 2 * c + 1) else 0
                    p2 = pss.tile([128, 2, CHUNK], F32, tag="s", name=f"s{j}")
                    for sl in range(2):
                        kt = 2 * j + sl
                        ms = 128 * (kt - 4 * c) if diag else 0
                        nc.tensor.matmul(
                            p2[:, sl, ms:],
                            qk_sb[base:base + 64, 2 + hp, kt * 128:(kt + 1) * 128],
                            qk_sb[base:base + 64, hp, bass.ds(c * CHUNK + ms, CHUNK - ms)],
                            start=True, stop=True)
                    nc.scalar.activation(p16[:, j, :, msp:], p2[:, :, msp:],
                                         EXP, bias=0.0, scale=EXP_SCALE)
                    if diag:
                        i0 = 2 * j - 4 * c  # 0 or 2
                        for sl in range(2):
                            col0 = 128 * (i0 + sl)
                            nc.gpsimd.tensor_tensor(
                                p16[:, j, sl, col0:col0 + 128],
                                p16[:, j, sl, col0:col0 + 128], tri_sb[:], MULT)
                            if sl == 1:
                                nc.gpsimd.memset(p16[:, j, 1, col0 - 128:col0], 0.0)
                return p16

            def emit_attn_B(c, hp, par, p16):
                """AV (bf16, per k-tile) + normalize -> y16."""
                cc = bass.ds(c * CHUNK, CHUNK)
                base = 64 * par
                nkt_c = 4 * c + 4
                nout = 65 if par == 0 else 128
                voff = 0 if par == 0 else 64
                pav = psav.tile([128, CHUNK], F32, tag="av", name=f"pav{par}")
                for kt in range(nkt_c):
                    i = kt - 4 * c
                    ms = 128 * i if i >= 0 else 0
                    nc.tensor.matmul(
                        pav[:nout, ms:], v16_sb[:, kt, hp, voff:voff + nout],
                        p16[:, kt // 2, kt % 2, ms:], start=(kt == 0),
                        stop=(kt == nkt_c - 1), skip_group_check=True)
                srow = 64 if par == 0 else 0
                yrows = 0 if par == 0 else 64
                rr_sb = rsmp.tile([1, CHUNK], F32R, tag="r")
                with nc.allow_low_precision(reason="1/Z for softmax, f32r is plenty"):
                    nc.vector.reciprocal(rr_sb[:], pav[srow:srow + 1, :])
                pbc = psmm.tile([128, CHUNK], F32, tag="mm", name="pbc")
                nc.tensor.matmul(pbc[:], ones_sb[:], rr_sb[:], start=True, stop=True)
                bc_sb = rsmp.tile([128, CHUNK], F32, tag="bc")
                nc.vector.tensor_copy(bc_sb[base:base + 64, :], pbc[base:base + 64, :])
                nc.vector.tensor_tensor(
                    y16_sb[base:base + 64, hp, cc], pav[yrows:yrows + 64, :],
                    bc_sb[base:base + 64, :], MULT)

            def emit_proj(c, tail=False):
                cc = bass.ds(c * CHUNK, CHUNK)
                o_sb = outsp.tile([128, 8, CHUNK], F16, tag="o")
                for db in range(8):
                    pr = psmm.tile([128, CHUNK], F32, tag="mm", name="pr")
                    for pt2 in range(2):
                        nc.tensor.matmul(
                            pr[:], wproj_sb[:, pt2, db * 128:(db + 1) * 128],
                            y16_sb[:, pt2, cc], start=(pt2 == 0), stop=(pt2 == 1))
                    if db % 2 == 0:
                        nc.vector.tensor_copy(o_sb[:, db], pr[:])
                    else:
                        nc.scalar.copy(o_sb[:, db], pr[:])
                    nc.sync.dma_start(
                        yT[db * 128:(db + 1) * 128, cc], o_sb[:, db])

            def emit_proj_tail_half(c, o_sb, prs):
                """First contraction half (pt2=0, hp0) of the last chunk's
                proj: depends only on B0/B1, runs during B3's norm chain.
                6 psum slots: 2 psmm + both halves of 2 pss tiles."""
                cc = bass.ds(c * CHUNK, CHUNK)
                for db in range(4):
                    if db % 2 == 0:
                        prs["t"] = pss.tile([128, 2, CHUNK], F32, tag="s", name="prs")
                        pr = prs["t"][:, 0]
                    else:
                        pr = prs["t"][:, 1]
                    nc.tensor.matmul(pr[:], wproj_sb[:, 0, db * 128:(db + 1) * 128],
                                     y16_sb[:, 0, cc], start=True, stop=False)
                    prs[db] = pr

            def emit_proj_tail(c, o_sb, prs):
                cc = bass.ds(c * CHUNK, CHUNK)
                for db in range(8):
                    if db < 4:
                        pr = prs[db]
                    else:
                        pr = psmm.tile([128, CHUNK], F32, tag="mm", name="pr")
                        nc.tensor.matmul(pr[:], wproj_sb[:, 0, db * 128:(db + 1) * 128],
                                         y16_sb[:, 0, cc], start=True, stop=False)
                    nc.tensor.matmul(pr[:], wproj_sb[:, 1, db * 128:(db + 1) * 128],
                                     y16_sb[:, 1, cc], start=False, stop=True)
                    # latency-optimized: halves on both engines
                    nc.vector.tensor_copy(o_sb[:, db, 0:256], pr[:, 0:256])
                    nc.scalar.copy(o_sb[:, db, 256:], pr[:, 256:])
                    nc.sync.dma_start(
                        yT[db * 128:(db + 1) * 128, cc], o_sb[:, db])

            # software pipeline: attn A/B interleaved; qkv(c+1) and rope(c+1)
            # emitted early inside chunk c so chunk c+1's S can start the
            # moment chunk c's EXP stream drains; proj delayed by half a chunk.
            BLKS = [(0, 0), (0, 1), (1, 0), (1, 1)]
            emit_qkv_qk(0, (0, 2))
            emit_rope(0, (0, 2), fast=True)
            emit_qkv_qk(0, (1, 3))
            emit_rope(0, (1, 3), fast=True)
            emit_qkv_v(0)
            # B3 of chunk c is delayed into chunk c+1's A-stream so its
            # EXP-tail waits overlap fresh S work instead of stalling PE.
            pend = None  # (chunk, p16 tile) for the delayed B3
            for c in range(NCH):
                p8s = {}
                p8s[0] = emit_attn_A(c, *BLKS[0])
                if c + 1 < NCH:
                    emit_qkv_qk(c + 1, (0, 2))
                p8s[1] = emit_attn_A(c, *BLKS[1])
                if pend is not None:
                    emit_attn_B(pend[0], *BLKS[3], pend[1])
                    emit_proj(pend[0])
                    pend = None
                if c + 1 < NCH:
                    emit_rope(c + 1, (0, 2))
                p8s[2] = emit_attn_A(c, *BLKS[2])
                emit_attn_B(c, *BLKS[0], p8s.pop(0))
                if c + 1 < NCH:
                    emit_qkv_qk(c + 1, (1, 3))
                p8s[3] = emit_attn_A(c, *BLKS[3])
                emit_attn_B(c, *BLKS[1], p8s.pop(1))
                if c + 1 < NCH:
                    emit_rope(c + 1, (1, 3))
                emit_attn_B(c, *BLKS[2], p8s.pop(2))
                if c + 1 < NCH:
                    emit_qkv_v(c + 1)
                pend = (c, p8s.pop(3))
            o_tail = outsp.tile([128, 8, CHUNK], F16, tag="o")
            prs = {}
            emit_proj_tail_half(NCH - 1, o_tail, prs)
            emit_attn_B(NCH - 1, *BLKS[3], pend[1])
            emit_proj_tail(NCH - 1, o_tail, prs)

    nc.finalize()
    return nc


def _host_inputs(x, Wqkv, bqkv, Wproj):
    """Per-core input maps. Core c: batch c//TP, heads [4*(c%TP), 4*(c%TP)+4)."""
    d = np.arange(64)
    inv_freq = 1.0 / (ROPE_BASE ** (np.arange(0, DH, 2, dtype=np.float64) / DH))  # [32]
    ang = np.arange(T, dtype=np.float64)[None, :] * inv_freq[d // 2][:, None]     # [64, T]
    cos64 = np.cos(ang)
    sin64 = np.sin(ang) * np.where(d % 2 == 0, -1.0, 1.0)[:, None]
    cos2 = np.tile(cos64, (2, 1)).astype(BF16NP)
    sin2 = np.tile(sin64, (2, 1)).astype(BF16NP)

    ki, qi = np.meshgrid(np.arange(128), np.arange(128), indexing="ij")
    tri16 = np.where(ki <= qi, 1.0, 0.0).astype(BF16NP)

    ones64 = _round_fp32r(np.ones((1, 128), np.float32))

    def f8_pair(a):
        a8 = a.astype(F8NP)
        ar8 = (a - a8.astype(np.float64)).astype(F8NP)
        return a8, ar8

    Wq, Wk, Wv = Wqkv[:, :D], Wqkv[:, D:2 * D], Wqkv[:, 2 * D:]
    bq, bk, bvv = bqkv[:D], bqkv[D:2 * D], bqkv[2 * D:]

    maps = []
    for core in range(NCORES):
        b, r = core // TP, core % TP
        sl = slice(r * DIN, (r + 1) * DIN)
        wqk_c = np.concatenate([Wq[:, sl], Wk[:, sl]], axis=1).astype(np.float64) * WS
        bqk_c = (np.concatenate([bq[sl], bk[sl]]) * WS).astype(np.float32)
        x8, xr8 = f8_pair(x[b].T.astype(np.float64))
        wqk8, wqkr8 = f8_pair(wqk_c)
        wv8, wvr8 = f8_pair(Wv[:, sl].astype(np.float64) * WS)
        maps.append({
            "xT8": x8, "xTr8": xr8,
            "wqk8": wqk8, "wqkr8": wqkr8,
            "wv8": wv8, "wvr8": wvr8,
            "wproj16": (Wproj[sl, :] * WS).astype(BF16NP),
            "bqk": np.ascontiguousarray(bqk_c.reshape(4, 128).T),
            "bv": np.broadcast_to((bvv[sl] * WS).astype(np.float32), (128, DIN)).copy(),
            "cos2": cos2,
            "sin2": sin2,
            "tri16": tri16,
            "ones64": ones64,
        })
    return maps


def kernel(x, Wqkv, bqkv, Wproj, bproj):
    global _compiled, _last_results
    from concourse.bass_utils import run_bass_kernel_spmd

    if _compiled is None:
        _compiled = _build()
    nc = _compiled

    maps = _host_inputs(
        np.asarray(x, np.float32), np.asarray(Wqkv, np.float32),
        np.asarray(bqkv, np.float32), np.asarray(Wproj, np.float32))
    res = run_bass_kernel_spmd(nc, maps, core_ids=list(range(NCORES)))
    _last_results = res
    out = np.empty((B, T, D), np.float32)
    for b in range(B):
        acc = np.zeros((D, T), np.float64)
        for r in range(TP):
            acc += res.results[b * TP + r]["yT"].astype(np.float64)
        out[b] = (acc / OUT_DIV).T + np.asarray(bproj, np.float64)[None, :]
    return out


# revision 52
# speedup vs baseline: 1.6501x; 1.0141x over previous
"""Causal self-attention (B=2, T=2048, D=1024, H=16, DH=64) on 8 trn2 cores.

Sharding: DP on batch (2) x TP on heads (4 heads/core). Each core computes
qkv for its heads from x[b]^T, RoPE, causal SDPA, and a partial row-parallel
output projection y^T [D, T] (f16, x1024 scale). Host sums TP partials,
rescales, transposes, adds bias.

v4 precision/engine economics (TimelineSim cost model):
- matmul cost = moving rows only; fp8e4 DoubleRow (2 K-slices/pass) is the
  only fast mode, but a single fp8 quantization injects ~3% relative error
  into a dot product (it does NOT average out), which blows the 2e-2 budget.
  QKV therefore runs fp8 DR with host-side error-feedback residuals
  (x8@W8 + xr8@W8 + x8@Wr8 -> ~0.4% error at 3/4 the bf16 cost); S, AV and
  proj run bf16 (1.0 cyc/row, no free-size penalty).
- Weights scaled x32 on host so 0.02-scale values stay in e4m3 normal range;
  S carries x1024 folded into the exp scale; v carries x32 into y and proj
  another x32 -> one final /1024 on host.
- softmax exp: one ACT instruction per PAIR of k-tiles over a 2-bank PSUM
  span, bf16 out; causal mask = 0/1 bf16 multiply on GpSimd (GpSimd cannot
  touch PSUM; only SBUF-resident work goes there). Normalizer Z rides row
  64/0 of the AV psum via a ones-column in V.
- rope pair-swap via one 128-partition stream_shuffle (mask applies per
  32-quadrant); bf16 tensors get the DVE 2x mode.
- 1/Z broadcast via PE ones-outer-product; DMAs batched (x loads, one f16
  y^T store per db+chunk, V-const regions via memset).
"""
import sys

if "/opt/trn_rl_repo" not in sys.path:
    sys.path.insert(0, "/opt/trn_rl_repo")

import numpy as np
import ml_dtypes

F8NP = ml_dtypes.float8_e4m3
BF16NP = ml_dtypes.bfloat16

B, T, D = 2, 2048, 1024
H, DH = 16, 64
ROPE_BASE = 10000.0
NCORES = 8
TP = 4                # TP group size (cores per batch)
HL = H // TP          # heads per core = 4
CHUNK = 512           # t/q chunk
NCH = T // CHUNK      # 4
NKT = T // 128        # 16 k-tiles
NJ = NKT // 2         # k-tile pairs = 8
DIN = HL * DH         # 256 local head dims
WS = 32.0             # host weight scale (Wqkv, Wv, Wproj)
SCALE = 1.0 / float(np.sqrt(DH))
EXP_SCALE = SCALE / (WS * WS)         # S carries WS^2
OUT_DIV = float(WS * WS)              # host divides final output

_compiled = None
_last_results = None


def _round_fp32r(x: np.ndarray) -> np.ndarray:
    u = np.ascontiguousarray(x, dtype=np.float32).view(np.uint32)
    u = (u + np.uint32(0x7FF) + ((u >> np.uint32(12)) & np.uint32(1))) & np.uint32(0xFFFFF000)
    return u.view(np.float32)


def _build(debug=False):
    import concourse.bass as bass
    import concourse.mybir as mybir
    import concourse.tile as tile
    from concourse import bacc

    F32 = mybir.dt.float32
    F32R = mybir.dt.float32r
    F16 = mybir.dt.float16
    BF16 = mybir.dt.bfloat16
    F8 = mybir.dt.float8e4
    ADD = mybir.AluOpType.add
    MULT = mybir.AluOpType.mult
    EXP = mybir.ActivationFunctionType.Exp
    DR = mybir.MatmulPerfMode.DoubleRow
    SWAP_MASK = [i ^ 1 for i in range(32)]

    nc = bacc.Bacc("TRN2", target_bir_lowering=False, num_devices=NCORES)

    xT8 = nc.dram_tensor("xT8", [D, T], F8, kind="ExternalInput")
    xTr8 = nc.dram_tensor("xTr8", [D, T], F8, kind="ExternalInput")
    wqk8 = nc.dram_tensor("wqk8", [D, 2 * DIN], F8, kind="ExternalInput")
    wqkr8 = nc.dram_tensor("wqkr8", [D, 2 * DIN], F8, kind="ExternalInput")
    wv8 = nc.dram_tensor("wv8", [D, DIN], F8, kind="ExternalInput")
    wvr8 = nc.dram_tensor("wvr8", [D, DIN], F8, kind="ExternalInput")
    wproj16 = nc.dram_tensor("wproj16", [DIN, D], BF16, kind="ExternalInput")
    bqk = nc.dram_tensor("bqk", [128, 4], F32, kind="ExternalInput")
    bv = nc.dram_tensor("bv", [128, DIN], F32, kind="ExternalInput")
    cos2 = nc.dram_tensor("cos2", [128, T], BF16, kind="ExternalInput")
    sin2 = nc.dram_tensor("sin2", [128, T], BF16, kind="ExternalInput")
    tri16 = nc.dram_tensor("tri16", [128, 128], BF16, kind="ExternalInput")
    ones64 = nc.dram_tensor("ones64", [1, 128], F32R, kind="ExternalInput")
    yT = nc.dram_tensor("yT", [D, T], F16, kind="ExternalOutput")

    with tile.TileContext(nc) as tc:
        with tc.tile_pool(name="const", bufs=1) as constp, \
             tc.tile_pool(name="big", bufs=1) as bigp, \
             tc.tile_pool(name="p8p", bufs=4) as p8p, \
             tc.tile_pool(name="tmp", bufs=3) as tmpp, \
             tc.tile_pool(name="rsm", bufs=2) as rsmp, \
             tc.tile_pool(name="outs", bufs=2) as outsp, \
             tc.tile_pool(name="psmm", bufs=2, space="PSUM") as psmm, \
             tc.tile_pool(name="pss", bufs=2, space="PSUM") as pss, \
             tc.tile_pool(name="psav", bufs=2, space="PSUM") as psav:

            # ---- persistent SBUF tensors ----
            wqk_sb = constp.tile([128, 8, 2 * DIN], F8)       # [p, o, f]
            wqkr_sb = constp.tile([128, 8, 2 * DIN], F8)
            wv_sb = constp.tile([128, 8, DIN], F8)
            wvr_sb = constp.tile([128, 8, DIN], F8)
            wproj_sb = constp.tile([128, 2, D], BF16)         # [p, pt2, dout]
            bqk_sb = constp.tile([128, 4], F32)
            bv_sb = constp.tile([128, DIN], F32)
            cos_sb = constp.tile([128, T], BF16)
            sin_sb = constp.tile([128, T], BF16)
            tri_sb = constp.tile([128, 128], BF16)
            ones_sb = constp.tile([1, 128], F32R)

            x_sb = bigp.tile([128, 8, T], F8)                 # all of x^T (fp8)
            xr_sb = bigp.tile([128, 8, T], F8)                # fp8 residual of x^T
            qk_sb = bigp.tile([128, 4, T], BF16)              # fb: q01,q23,k01,k23
            v16_sb = bigp.tile([128, NKT, 2, 192], BF16)      # [kpos, kt, hp, cols]
            y16_sb = bigp.tile([128, 2, T], BF16)             # y^T bf16 (x32 scale)

            # startup DMAs, batched; chunk-0 deps first so qkv can start early
            def xload(c):
                for t_sb, t_dr in ((x_sb, xT8), (xr_sb, xTr8)):
                    nc.sync.dma_start(
                        t_sb[:, :, c * CHUNK:(c + 1) * CHUNK],
                        t_dr[:].rearrange("(o p) t -> p o t", p=128)[
                            :, :, c * CHUNK:(c + 1) * CHUNK])

            wqk4 = wqk8[:].rearrange("(o p) (fb f) -> p o fb f", p=128, f=128)
            wqkr4 = wqkr8[:].rearrange("(o p) (fb f) -> p o fb f", p=128, f=128)
            wqk_sb4 = wqk_sb[:].rearrange("p o (fb f) -> p o fb f", f=128)
            wqkr_sb4 = wqkr_sb[:].rearrange("p o (fb f) -> p o fb f", f=128)
            # DMAs ordered by first-use time; first-matmul deps split small
            xT3 = xT8[:].rearrange("(o p) t -> p o t", p=128)
            xTr3 = xTr8[:].rearrange("(o p) t -> p o t", p=128)
            nc.sync.dma_start(x_sb[:, 0:2, 0:CHUNK], xT3[:, 0:2, 0:CHUNK])
            nc.sync.dma_start(wqk_sb4[:, :, 0], wqk4[:, :, 0])
            nc.sync.dma_start(x_sb[:, 2:8, 0:CHUNK], xT3[:, 2:8, 0:CHUNK])
            nc.sync.dma_start(wqk_sb4[:, :, 2], wqk4[:, :, 2])
            nc.sync.dma_start(xr_sb[:, :, 0:CHUNK], xTr3[:, :, 0:CHUNK])
            for fb in (0, 2):
                nc.sync.dma_start(wqkr_sb4[:, :, fb], wqkr4[:, :, fb])
            nc.sync.dma_start(bqk_sb[:], bqk[:])
            nc.sync.dma_start(cos_sb[:], cos2[:])
            nc.sync.dma_start(sin_sb[:], sin2[:])
            for fb in (1, 3):
                nc.sync.dma_start(wqk_sb4[:, :, fb], wqk4[:, :, fb])
                nc.sync.dma_start(wqkr_sb4[:, :, fb], wqkr4[:, :, fb])
            nc.sync.dma_start(wv_sb[:], wv8[:].rearrange("(o p) f -> p o f", p=128))
            nc.sync.dma_start(wvr_sb[:], wvr8[:].rearrange("(o p) f -> p o f", p=128))
            nc.sync.dma_start(bv_sb[:], bv[:])
            xload(1)
            nc.sync.dma_start(tri_sb[:], tri16[:])
            nc.sync.dma_start(ones_sb[:], ones64[:])
            nc.sync.dma_start(wproj_sb[:], wproj16[:].rearrange("(o p) f -> p o f", p=128))
            xload(2)
            xload(3)

            # V const regions: col 64 = 1.0, cols 65:128 = 0.0 (per kt, hp)
            nc.gpsimd.memset(v16_sb[:, :, :, 64:65], 1.0)
            nc.gpsimd.memset(v16_sb[:, :, :, 65:128], 0.0)

            # dummy EXP so the ACT table loads during startup DMAs
            scratch = tmpp.tile([128, 1], F32, tag="warm")
            nc.scalar.activation(scratch[:], bqk_sb[:, 0:1], EXP, bias=0.0, scale=0.0)

            def emit_qkv_qk(c, fbs=(0, 1, 2, 3)):
                cc = bass.ds(c * CHUNK, CHUNK)
                xc, xrc = x_sb[:, :, cc], xr_sb[:, :, cc]
                # q^T,k^T: fp8 DR with residual terms, accumulated in PSUM
                for fb in fbs:
                    fsl = bass.ds(fb * 128, 128)
                    pq = psmm.tile([128, CHUNK], F32, tag="mm", name="pq")
                    for j in range(4):
                        nc.tensor.matmul(pq[:], wqk_sb[:, 2 * j:2 * j + 2, fsl],
                                         xc[:, 2 * j:2 * j + 2],
                                         start=(j == 0), stop=False, perf_mode=DR)
                    for j in range(4):
                        nc.tensor.matmul(pq[:], wqk_sb[:, 2 * j:2 * j + 2, fsl],
                                         xrc[:, 2 * j:2 * j + 2],
                                         start=False, stop=False, perf_mode=DR)
                    for j in range(4):
                        nc.tensor.matmul(pq[:], wqkr_sb[:, 2 * j:2 * j + 2, fsl],
                                         xc[:, 2 * j:2 * j + 2],
                                         start=False, stop=(j == 3), perf_mode=DR)
                    nc.vector.tensor_scalar_add(qk_sb[:, fb, cc], pq[:], bqk_sb[:, fb:fb + 1])

            def emit_qkv_v(c):
                cc = bass.ds(c * CHUNK, CHUNK)
                # V for this chunk (natural layout [t, f])
                for tb in range(4):
                    kt = c * 4 + tb
                    tsl = bass.ds(tb * 128, 128)
                    pvfull = psmm.tile([128, CHUNK], F32, tag="mm", name="pvfull")
                    pv = pvfull[:, :DIN]
                    for j in range(4):
                        nc.tensor.matmul(pv[:], x_sb[:, 2 * j:2 * j + 2, cc][:, :, tsl],
                                         wv_sb[:, 2 * j:2 * j + 2],
                                         start=(j == 0), stop=False, perf_mode=DR)
                    for j in range(4):
                        nc.tensor.matmul(pv[:], xr_sb[:, 2 * j:2 * j + 2, cc][:, :, tsl],
                                         wv_sb[:, 2 * j:2 * j + 2],
                                         start=False, stop=False, perf_mode=DR)
                    for j in range(4):
                        nc.tensor.matmul(pv[:], x_sb[:, 2 * j:2 * j + 2, cc][:, :, tsl],
                                         wvr_sb[:, 2 * j:2 * j + 2],
                                         start=False, stop=(j == 3), perf_mode=DR)
                    for par in range(2):
                        # even heads l=0,2 -> hp 0,1 cols 0:64 ; odd l=1,3 -> 128:192
                        nc.vector.tensor_tensor(
                            v16_sb[:, kt, :, 128 * par:128 * par + 64],
                            pv[:].rearrange("p (l d) -> p l d", d=64)[:, par::2],
                            bv_sb[:].rearrange("p (l d) -> p l d", d=64)[:, par::2],
                            ADD)

            def emit_rope(c, fbs=(0, 1, 2, 3), fast=False):
                # fast=True keeps the whole chain on DVE (bf16 2x) to minimize
                # latency when other engines are idle (startup)
                eng2 = nc.vector if fast else nc.gpsimd
                cc = bass.ds(c * CHUNK, CHUNK)
                for fb in fbs:
                    swp = tmpp.tile([128, CHUNK], BF16, tag="rope")
                    nc.vector.stream_shuffle(swp[:], qk_sb[:, fb, cc], SWAP_MASK)
                    nc.vector.tensor_tensor(swp[:], swp[:], sin_sb[:, cc], MULT)
                    eng2.tensor_tensor(qk_sb[:, fb, cc], qk_sb[:, fb, cc],
                                       cos_sb[:, cc], MULT)
                    eng2.tensor_tensor(qk_sb[:, fb, cc], qk_sb[:, fb, cc],
                                       swp[:], ADD)

            def emit_attn_A(c, hp, par):
                """S pairs + EXP + mask -> p16 tile; returns the tile."""
                base = 64 * par
                nj_c = 2 * c + 2
                p16 = p8p.tile([128, NJ, 2, CHUNK], BF16, tag="p8", name=f"p16_{hp}{par}")
                for j in range(nj_c):
                    diag = (j >= 2 * c)
                    msp = 256 if (diag and j == 2 * c + 1) else 0
                    p2 = pss.tile([128, 2, CHUNK], F32, tag="s", name=f"s{j}")
                    for sl in range(2):
                        kt = 2 * j + sl
                        ms = 128 * (kt - 4 * c) if diag else 0
                        nc.tensor.matmul(
                            p2[:, sl, ms:],
                            qk_sb[base:base + 64, 2 + hp, kt * 128:(kt + 1) * 128],
                            qk_sb[base:base + 64, hp, bass.ds(c * CHUNK + ms, CHUNK - ms)],
                            start=True, stop=True)
                    nc.scalar.activation(p16[:, j, :, msp:], p2[:, :, msp:],
                                         EXP, bias=0.0, scale=EXP_SCALE)
                    if diag:
                        i0 = 2 * j - 4 * c  # 0 or 2
                        for sl in range(2):
                            col0 = 128 * (i0 + sl)
                            nc.gpsimd.tensor_tensor(
                                p16[:, j, sl, col0:col0 + 128],
                                p16[:, j, sl, col0:col0 + 128], tri_sb[:], MULT)
                            if sl == 1:
                                nc.gpsimd.memset(p16[:, j, 1, col0 - 128:col0], 0.0)
                return p16

            def emit_attn_B(c, hp, par, p16):
                """AV (bf16, per k-tile) + normalize -> y16."""
                cc = bass.ds(c * CHUNK, CHUNK)
                base = 64 * par
                nkt_c = 4 * c + 4
                nout = 65 if par == 0 else 128
                voff = 0 if par == 0 else 64
                pav = psav.tile([128, CHUNK], F32, tag="av", name=f"pav{par}")
                for kt in range(nkt_c):
                    i = kt - 4 * c
                    ms = 128 * i if i >= 0 else 0
                    nc.tensor.matmul(
                        pav[:nout, ms:], v16_sb[:, kt, hp, voff:voff + nout],
                        p16[:, kt // 2, kt % 2, ms:], start=(kt == 0),
                        stop=(kt == nkt_c - 1), skip_group_check=True)
                srow = 64 if par == 0 else 0
                yrows = 0 if par == 0 else 64
                rr_sb = rsmp.tile([1, CHUNK], F32R, tag="r")
                with nc.allow_low_precision(reason="1/Z for softmax, f32r is plenty"):
                    nc.vector.reciprocal(rr_sb[:], pav[srow:srow + 1, :])
                pbc = psmm.tile([128, CHUNK], F32, tag="mm", name="pbc")
                nc.tensor.matmul(pbc[:], ones_sb[:], rr_sb[:], start=True, stop=True)
                bc_sb = rsmp.tile([128, CHUNK], F32, tag="bc")
                nc.vector.tensor_copy(bc_sb[base:base + 64, :], pbc[base:base + 64, :])
                nc.vector.tensor_tensor(
                    y16_sb[base:base + 64, hp, cc], pav[yrows:yrows + 64, :],
                    bc_sb[base:base + 64, :], MULT)

            def emit_proj(c, dve_only=False):
                cc = bass.ds(c * CHUNK, CHUNK)
                o_sb = outsp.tile([128, 8, CHUNK], F16, tag="o")
                for db in range(8):
                    pr = psmm.tile([128, CHUNK], F32, tag="mm", name="pr")
                    for pt2 in range(2):
                        nc.tensor.matmul(
                            pr[:], wproj_sb[:, pt2, db * 128:(db + 1) * 128],
                            y16_sb[:, pt2, cc], start=(pt2 == 0), stop=(pt2 == 1))
                    if db % 2 == 0 or dve_only:
                        nc.vector.tensor_copy(o_sb[:, db], pr[:])
                    else:
                        nc.scalar.copy(o_sb[:, db], pr[:])
                    nc.sync.dma_start(
                        yT[db * 128:(db + 1) * 128, cc], o_sb[:, db])

            def emit_proj_tail_early(c, o_sb, prs):
                """Last-chunk proj, pieces that don't need B3: pt2=0 (hp0,
                from B0/B1) and the par0 half of pt2=1 (partitions 0:64, from
                B2). dbs 0-3 use both halves of 2 pss tiles, db 4 one psmm
                slot; the other psmm slot stays free for B3's pbc."""
                cc = bass.ds(c * CHUNK, CHUNK)
                for db in range(4):
                    if db % 2 == 0:
                        prs["t"] = pss.tile([128, 2, CHUNK], F32, tag="s", name="prs")
                        pr = prs["t"][:, 0]
                    else:
                        pr = prs["t"][:, 1]
                    dsl = bass.ds(db * 128, 128)
                    nc.tensor.matmul(pr[:], wproj_sb[:, 0, dsl], y16_sb[:, 0, cc],
                                     start=True, stop=False, skip_group_check=True)
                    prs[db] = pr

            def emit_proj_tail_late(c, o_sb, prs):
                cc = bass.ds(c * CHUNK, CHUNK)
                for db in range(8):
                    dsl = bass.ds(db * 128, 128)
                    if db < 4:
                        pr = prs[db]
                        nc.tensor.matmul(pr[:], wproj_sb[:, 1, dsl],
                                         y16_sb[:, 1, cc],
                                         start=False, stop=True, skip_group_check=True)
                    else:
                        pr = psmm.tile([128, CHUNK], F32, tag="mm", name="pr")
                        for pt2 in range(2):
                            nc.tensor.matmul(pr[:], wproj_sb[:, pt2, dsl],
                                             y16_sb[:, pt2, cc],
                                             start=(pt2 == 0), stop=(pt2 == 1))
                    # latency-optimized: halves on both engines
                    nc.vector.tensor_copy(o_sb[:, db, 0:256], pr[:, 0:256])
                    nc.scalar.copy(o_sb[:, db, 256:], pr[:, 256:])
                    nc.sync.dma_start(
                        yT[db * 128:(db + 1) * 128, cc], o_sb[:, db])

            # software pipeline: attn A/B interleaved; qkv(c+1) and rope(c+1)
            # emitted early inside chunk c so chunk c+1's S can start the
            # moment chunk c's EXP stream drains; proj delayed by half a chunk.
            BLKS = [(0, 0), (0, 1), (1, 0), (1, 1)]
            emit_qkv_qk(0, (0, 2))
            emit_rope(0, (0, 2), fast=True)
            emit_qkv_qk(0, (1, 3))
            emit_rope(0, (1, 3), fast=True)
            emit_qkv_v(0)
            # B3 of chunk c is delayed into chunk c+1's A-stream so its
            # EXP-tail waits overlap fresh S work instead of stalling PE.
            pend = None  # (chunk, p16 tile) for the delayed B3
            for c in range(NCH):
                p8s = {}
                p8s[0] = emit_attn_A(c, *BLKS[0])
                if c + 1 < NCH:
                    emit_qkv_qk(c + 1, (0, 2))
                p8s[1] = emit_attn_A(c, *BLKS[1])
                if pend is not None:
                    emit_attn_B(pend[0], *BLKS[3], pend[1])
                    if c < NCH - 1:
                        emit_proj(pend[0])
                    pend = None
                if c + 1 < NCH:
                    emit_rope(c + 1, (0, 2))
                p8s[2] = emit_attn_A(c, *BLKS[2])
                emit_attn_B(c, *BLKS[0], p8s.pop(0))
                if c + 1 < NCH:
                    emit_qkv_qk(c + 1, (1, 3))
                p8s[3] = emit_attn_A(c, *BLKS[3])
                if c == NCH - 1:
                    # late filler: proj(c-1) lands under A3's EXP drain
                    emit_proj(c - 1, dve_only=True)
                emit_attn_B(c, *BLKS[1], p8s.pop(1))
                if c + 1 < NCH:
                    emit_rope(c + 1, (1, 3))
                emit_attn_B(c, *BLKS[2], p8s.pop(2))
                if c + 1 < NCH:
                    emit_qkv_v(c + 1)
                pend = (c, p8s.pop(3))
            o_tail = outsp.tile([128, 8, CHUNK], F16, tag="o")
            prs = {}
            emit_proj_tail_early(NCH - 1, o_tail, prs)
            emit_attn_B(NCH - 1, *BLKS[3], pend[1])
            emit_proj_tail_late(NCH - 1, o_tail, prs)

    nc.finalize()
    return nc


def _host_inputs(x, Wqkv, bqkv, Wproj):
    """Per-core input maps. Core c: batch c//TP, heads [4*(c%TP), 4*(c%TP)+4)."""
    d = np.arange(64)
    inv_freq = 1.0 / (ROPE_BASE ** (np.arange(0, DH, 2, dtype=np.float64) / DH))  # [32]
    ang = np.arange(T, dtype=np.float64)[None, :] * inv_freq[d // 2][:, None]     # [64, T]
    cos64 = np.cos(ang)
    sin64 = np.sin(ang) * np.where(d % 2 == 0, -1.0, 1.0)[:, None]
    cos2 = np.tile(cos64, (2, 1)).astype(BF16NP)
    sin2 = np.tile(sin64, (2, 1)).astype(BF16NP)

    ki, qi = np.meshgrid(np.arange(128), np.arange(128), indexing="ij")
    tri16 = np.where(ki <= qi, 1.0, 0.0).astype(BF16NP)

    ones64 = _round_fp32r(np.ones((1, 128), np.float32))

    def f8_pair(a):
        a8 = a.astype(F8NP)
        ar8 = (a - a8.astype(np.float64)).astype(F8NP)
        return a8, ar8

    Wq, Wk, Wv = Wqkv[:, :D], Wqkv[:, D:2 * D], Wqkv[:, 2 * D:]
    bq, bk, bvv = bqkv[:D], bqkv[D:2 * D], bqkv[2 * D:]

    maps = []
    for core in range(NCORES):
        b, r = core // TP, core % TP
        sl = slice(r * DIN, (r + 1) * DIN)
        wqk_c = np.concatenate([Wq[:, sl], Wk[:, sl]], axis=1).astype(np.float64) * WS
        bqk_c = (np.concatenate([bq[sl], bk[sl]]) * WS).astype(np.float32)
        x8, xr8 = f8_pair(x[b].T.astype(np.float64))
        wqk8, wqkr8 = f8_pair(wqk_c)
        wv8, wvr8 = f8_pair(Wv[:, sl].astype(np.float64) * WS)
        maps.append({
            "xT8": x8, "xTr8": xr8,
            "wqk8": wqk8, "wqkr8": wqkr8,
            "wv8": wv8, "wvr8": wvr8,
            "wproj16": (Wproj[sl, :] * WS).astype(BF16NP),
            "bqk": np.ascontiguousarray(bqk_c.reshape(4, 128).T),
            "bv": np.broadcast_to((bvv[sl] * WS).astype(np.float32), (128, DIN)).copy(),
            "cos2": cos2,
            "sin2": sin2,
            "tri16": tri16,
            "ones64": ones64,
        })
    return maps


def kernel(x, Wqkv, bqkv, Wproj, bproj):
    global _compiled, _last_results
    from concourse.bass_utils import run_bass_kernel_spmd

    if _compiled is None:
        _compiled = _build()
    nc = _compiled

    maps = _host_inputs(
        np.asarray(x, np.float32), np.asarray(Wqkv, np.float32),
        np.asarray(bqkv, np.float32), np.asarray(Wproj, np.float32))
    res = run_bass_kernel_spmd(nc, maps, core_ids=list(range(NCORES)))
    _last_results = res
    out = np.empty((B, T, D), np.float32)
    for b in range(B):
        acc = np.zeros((D, T), np.float64)
        for r in range(TP):
            acc += res.results[b * TP + r]["yT"].astype(np.float64)
        out[b] = (acc / OUT_DIV).T + np.asarray(bproj, np.float64)[None, :]
    return out
